# revision 18
# baseline (speedup 1.0000x reference)
"""CenterLoss (vq_codebook) Trainium2 kernel.

Computation (reference):
    batch_centers = centers[labels]                 # [B, D]
    center_loss   = mean_i ||features_i - batch_centers_i||^2
    counts        = segment_sum(ones, labels)       # [C]
    sums          = segment_sum(features, labels)   # [C, D]
    new_centers   = centers + where(counts>0, ALPHA*(sums/counts - centers), 0)

Strategy (8 NeuronCores, class-parallel):
  - Shard the 50000-row centers table row-wise: 6400 classes/core (50 tiles
    of 128 classes), padded to 51200 rows total.
  - Host routes (features, labels) by label shard ("all-to-all" done host
    side while sharding), grouping features per 128-class tile with <=64
    features per tile (two tiles packed per 128 SBUF partitions), and
    encodes labels as one-hot routing matrices.
  - Device (per core, per class tile t):
      sums_t [128,512]  = onehot_t.T @ feat_t      (PE)
      counts_t [128,1]  = onehot_t.T @ ones        (PE)
      new_t             = C_t*(1-a*present) + sums_t*(a*present/counts)  (ACT+DVE)
      loss parts: rownorm2(C_t) (ACT, fused accum), 2*sum(C_t*sums_t) (DVE ttr),
                  sum(feat^2) (ACT, fused accum)
    using loss = sum||f||^2 + sum_c (n_c*||c_c||^2 - 2*sums_c . c_c), all
    accumulated per-partition; host sums partials and divides by B.
  - Host gathers: concat new-center shards, trim to 50000; sum scalar loss.
"""

import numpy as np

FEAT_DIM = 512
NUM_CLASSES = 50000
ALPHA = 0.1
BATCH = 8192
NCORES = 8
TILES = 50                # class tiles of 128 per core
CSH = TILES * 128         # 6400 classes per core
CPAD = NCORES * CSH       # 51200
PAIRS = TILES // 2        # two class tiles share one 128-partition group
GSLOT = 64                # feature slots per class tile (half mode)

_CACHE = {}


def _build(feat_np_dtype, gp):
    """Build the per-core Bass program.

    gp=2: two class tiles per 128-partition feature group (64 slots each).
    gp=1: one class tile per group (128 slots) - fallback for dense tiles.

    Per class tile t: sums via one-hot matmul (bf16), per-feature center
    gather via transposed one-hot matmul (f32r), EMA update fused as
    ct*s1 + sums*s2, loss as sum((gathered - f)^2) per feature group.
    """
    import concourse.bacc as bacc
    import concourse.mybir as mybir
    import concourse.tile as tile
    from concourse.alu_op_type import AluOpType as op
    from contextlib import ExitStack

    f32 = mybir.dt.float32
    f32r = mybir.dt.float32r
    fdt = mybir.dt.from_np(np.dtype(feat_np_dtype))
    AF = mybir.ActivationFunctionType
    AX = mybir.AxisListType

    ngrp = TILES // gp                 # feature groups
    gslot = 128 // gp                  # slots per class tile
    ohcols = TILES * 128 + 1           # +1: trailing all-ones column

    nc = bacc.Bacc("TRN2", target_bir_lowering=False, debug=False)
    centers = nc.dram_tensor("centers", [TILES, 128, FEAT_DIM], f32r,
                             kind="ExternalInput").ap()
    featp = nc.dram_tensor("featp", [ngrp, gslot, gp * FEAT_DIM], fdt,
                           kind="ExternalInput").ap()
    ohp = nc.dram_tensor("ohp", [gslot, ohcols], fdt,
                         kind="ExternalInput").ap()
    ohg = nc.dram_tensor("ohg", [128, TILES * gslot], f32r,
                         kind="ExternalInput").ap()
    outc = nc.dram_tensor("outc", [TILES, 128, FEAT_DIM], f32,
                          kind="ExternalOutput").ap()
    lossp = nc.dram_tensor("lossp", [gslot, 1], f32,
                           kind="ExternalOutput").ap()

    with tile.TileContext(nc) as tc, ExitStack() as ctx:
        const = ctx.enter_context(tc.tile_pool(name="const", bufs=1))
        cpool = ctx.enter_context(tc.tile_pool(name="cts", bufs=5))
        fpool = ctx.enter_context(tc.tile_pool(name="fg", bufs=4))
        apool = ctx.enter_context(tc.tile_pool(name="aa", bufs=4))
        opool = ctx.enter_context(tc.tile_pool(name="oo", bufs=5))
        dpool = ctx.enter_context(tc.tile_pool(name="dd", bufs=3))
        spool = ctx.enter_context(tc.tile_pool(name="ss", bufs=2))
        pspool = ctx.enter_context(tc.tile_pool(name="psums", bufs=4, space="PSUM"))
        pgpool = ctx.enter_context(tc.tile_pool(name="psumg", bufs=3, space="PSUM"))
        pcpool = ctx.enter_context(tc.tile_pool(name="psumc", bufs=1, space="PSUM"))

        oh_all = const.tile([gslot, ohcols], fdt, tag="oh")
        nc.sync.dma_start(oh_all[:], ohp[:])
        ones = oh_all[:, ohcols - 1:ohcols]
        ohg_all = const.tile([128, TILES * gslot], f32r, tag="ohg")
        nc.sync.dma_start(ohg_all[:], ohg[:])

        def oh_tile(t):
            return oh_all[:, t * 128:(t + 1) * 128]

        # --- phase 1: per-class counts for all tiles, then scale vectors ---
        pc = pcpool.tile([128, TILES], f32, tag="pc")
        for t in range(TILES):
            nc.tensor.matmul(pc[:, t:t + 1], oh_tile(t), ones,
                             start=True, stop=True)
        counts = const.tile([128, TILES], f32, tag="counts")
        nc.vector.tensor_copy(counts[:], pc[:])
        mask = const.tile([128, TILES], f32, tag="mask")
        nc.vector.tensor_scalar(mask[:], counts[:], 0.5, None, op.is_ge)
        csafe = const.tile([128, TILES], f32, tag="csafe")
        nc.vector.tensor_scalar(csafe[:], counts[:], 1.0, None, op.max)
        rcp = const.tile([128, TILES], f32, tag="rcp")
        nc.vector.reciprocal(rcp[:], csafe[:])
        s1 = const.tile([128, TILES], f32, tag="s1")
        nc.vector.tensor_scalar(s1[:], mask[:], -ALPHA, 1.0, op.mult, op.add)
        s2 = const.tile([128, TILES], f32, tag="s2")
        nc.vector.scalar_tensor_tensor(s2[:], rcp[:], ALPHA, mask[:],
                                       op.mult, op.mult)

        lcol = const.tile([gslot, TILES], f32, tag="lcol")

        # --- phase 2: stream center tiles, one feature group per gp tiles ---
        for g in range(ngrp):
            fg = fpool.tile([gslot, gp * FEAT_DIM], fdt, tag="fg")
            nc.sync.dma_start(fg[:], featp[g])
            for h in range(gp):
                t = g * gp + h
                fgh = fg[:, h * FEAT_DIM:(h + 1) * FEAT_DIM]
                ct = cpool.tile([128, FEAT_DIM], f32r, tag="ct")
                nc.sync.dma_start(ct[:], centers[t])
                ps = pspool.tile([128, FEAT_DIM], f32, tag="ps")
                nc.tensor.matmul(ps[:], oh_tile(t), fgh,
                                 start=True, stop=True)
                psg = pgpool.tile([gslot, FEAT_DIM], f32, tag="psg")
                nc.tensor.matmul(psg[:],
                                 ohg_all[:, t * gslot:(t + 1) * gslot],
                                 ct[:], start=True, stop=True)
                aa = apool.tile([128, FEAT_DIM], f32, tag="aa")
                nc.gpsimd.tensor_scalar(aa[:], ct[:].bitcast(f32),
                                        s1[:, t:t + 1], None, op.mult)
                ot = opool.tile([128, FEAT_DIM], f32, tag="ot")
                nc.vector.scalar_tensor_tensor(ot[:], ps[:], s2[:, t:t + 1],
                                               aa[:], op.mult, op.add)
                nc.sync.dma_start(outc[t], ot[:])
                dt_ = dpool.tile([gslot, FEAT_DIM], f32, tag="dd")
                nc.vector.tensor_tensor(dt_[:], psg[:], fgh, op.subtract)
                sq = spool.tile([gslot, FEAT_DIM], f32, tag="sq")
                nc.scalar.activation(sq[:], dt_[:], AF.Square,
                                     accum_out=lcol[:, t:t + 1])

        # --- phase 3: fold loss partials to one [gslot,1] vector ---
        lv = const.tile([gslot, 1], f32, tag="lv")
        nc.vector.tensor_reduce(lv[:], lcol[:], AX.X, op.add)
        nc.sync.dma_start(lossp[:], lv[:])

    nc.compile()
    return nc


def _route(features, labels, feat_np_dtype, gp):
    """Host-side all-to-all: per core, group features by class tile and
    build one-hot routing matrices. Returns per-core (featp, ohp, ohg)."""
    ngrp = TILES // gp
    gslot = 128 // gp
    feat_maps = []
    oh_maps = []
    ohg_maps = []
    core = labels // CSH
    within = labels % CSH
    tl = within // 128          # class tile within core
    cc = within % 128           # class row within tile
    for m in range(NCORES):
        sel = np.nonzero(core == m)[0]
        t = tl[sel]
        order = np.argsort(t, kind="stable")
        sel, t = sel[order], t[order]
        c = cc[sel]
        # rank within each tile
        slot = np.arange(len(t)) - np.searchsorted(t, t)
        if len(slot) and slot.max() >= gslot:
            raise OverflowError("tile feature count exceeds slots")
        g, h = np.divmod(t, gp)
        featp = np.zeros((ngrp, gslot, gp, FEAT_DIM), dtype=feat_np_dtype)
        featp[g, slot, h] = features[sel]
        featp = featp.reshape(ngrp, gslot, gp * FEAT_DIM)
        ohp = np.zeros((gslot, TILES * 128 + 1), dtype=feat_np_dtype)
        ohp[slot, t * 128 + c] = 1.0
        ohp[:, -1] = 1.0
        ohg = np.zeros((128, TILES * gslot), dtype=np.float32)
        ohg[c, t * gslot + slot] = 1.0
        feat_maps.append(featp)
        oh_maps.append(ohp)
        ohg_maps.append(ohg)
    return feat_maps, oh_maps, ohg_maps


def _run(features, labels, centers, feat_np_dtype=None, trace=False):
    import ml_dtypes
    from concourse.bass_utils import run_bass_kernel_spmd

    if feat_np_dtype is None:
        feat_np_dtype = ml_dtypes.bfloat16

    features = np.ascontiguousarray(np.asarray(features, dtype=np.float32))
    labels = np.asarray(labels).astype(np.int64)
    centers = np.ascontiguousarray(np.asarray(centers, dtype=np.float32))

    centers_pad = np.zeros((CPAD, FEAT_DIM), dtype=np.float32)
    centers_pad[:NUM_CLASSES] = centers
    centers_pad = centers_pad.reshape(NCORES, TILES, 128, FEAT_DIM)

    try:
        gp = 2
        feat_maps, oh_maps, ohg_maps = _route(features, labels,
                                              feat_np_dtype, gp)
    except OverflowError:
        gp = 1
        feat_maps, oh_maps, ohg_maps = _route(features, labels,
                                              feat_np_dtype, gp)

    key = (np.dtype(feat_np_dtype).name, gp)
    if key not in _CACHE:
        _CACHE[key] = _build(feat_np_dtype, gp)
    nc = _CACHE[key]

    in_maps = [
        {"centers": np.ascontiguousarray(centers_pad[m]),
         "featp": feat_maps[m], "ohp": oh_maps[m], "ohg": ohg_maps[m]}
        for m in range(NCORES)
    ]
    res = run_bass_kernel_spmd(nc, in_maps, list(range(NCORES)), trace=trace)

    new_centers = np.concatenate(
        [res.results[m]["outc"].reshape(CSH, FEAT_DIM) for m in range(NCORES)],
        axis=0)[:NUM_CLASSES]
    loss = np.float32(
        sum(np.float64(res.results[m]["lossp"]).sum() for m in range(NCORES))
        / BATCH)
    return (loss, new_centers), res


def kernel(features, labels, centers):
    out, _ = _run(features, labels, centers)
    return out


# revision 19
# speedup vs baseline: 2.6981x; 2.6981x over previous
"""CenterLoss (vq_codebook) Trainium2 kernel.

Computation (reference):
    batch_centers = centers[labels]                 # [B, D]
    center_loss   = mean_i ||features_i - batch_centers_i||^2
    counts        = segment_sum(ones, labels)       # [C]
    sums          = segment_sum(features, labels)   # [C, D]
    new_centers   = centers + where(counts>0, ALPHA*(sums/counts - centers), 0)

Strategy (8 NeuronCores, class-parallel):
  - Shard the 50000-row centers table row-wise: 6400 classes/core (50 tiles
    of 128 classes), padded to 51200 rows total.
  - Host routes (features, labels) by label shard ("all-to-all" done host
    side while sharding), grouping features per 128-class tile with <=64
    features per tile (two tiles packed per 128 SBUF partitions), and
    encodes labels as one-hot routing matrices.
  - Device (per core, per class tile t):
      sums_t [128,512]  = onehot_t.T @ feat_t      (PE)
      counts_t [128,1]  = onehot_t.T @ ones        (PE)
      new_t             = C_t*(1-a*present) + sums_t*(a*present/counts)  (ACT+DVE)
      loss parts: rownorm2(C_t) (ACT, fused accum), 2*sum(C_t*sums_t) (DVE ttr),
                  sum(feat^2) (ACT, fused accum)
    using loss = sum||f||^2 + sum_c (n_c*||c_c||^2 - 2*sums_c . c_c), all
    accumulated per-partition; host sums partials and divides by B.
  - Host gathers: concat new-center shards, trim to 50000; sum scalar loss.
"""

import numpy as np

FEAT_DIM = 512
NUM_CLASSES = 50000
ALPHA = 0.1
BATCH = 8192
NCORES = 8
TILES = 50                # class tiles of 128 per core
CSH = TILES * 128         # 6400 classes per core
CPAD = NCORES * CSH       # 51200
PAIRS = TILES // 2        # two class tiles share one 128-partition group
GSLOT = 64                # feature slots per class tile (half mode)

_CACHE = {}


def _build(feat_np_dtype, gp):
    """Build the per-core Bass program.

    gp=2: two class tiles per 128-partition feature group (64 slots each).
    gp=1: one class tile per group (128 slots) - fallback for dense tiles.

    Per class tile t: sums via one-hot matmul (bf16), per-feature center
    gather via transposed one-hot matmul (f32r), EMA update fused as
    ct*s1 + sums*s2, loss as sum((gathered - f)^2) per feature group.
    """
    import concourse.bacc as bacc
    import concourse.mybir as mybir
    import concourse.tile as tile
    from concourse.alu_op_type import AluOpType as op
    from contextlib import ExitStack

    f32 = mybir.dt.float32
    f32r = mybir.dt.float32r
    fdt = mybir.dt.from_np(np.dtype(feat_np_dtype))
    AF = mybir.ActivationFunctionType
    AX = mybir.AxisListType

    ngrp = TILES // gp                 # feature groups
    gslot = 128 // gp                  # slots per class tile
    ohcols = TILES * 128 + 1           # +1: trailing all-ones column

    nc = bacc.Bacc("TRN2", target_bir_lowering=False, debug=False)
    centers = nc.dram_tensor("centers", [TILES, 128, FEAT_DIM], f32r,
                             kind="ExternalInput").ap()
    featp = nc.dram_tensor("featp", [ngrp, gslot, gp * FEAT_DIM], fdt,
                           kind="ExternalInput").ap()
    ohp = nc.dram_tensor("ohp", [gslot, ohcols], fdt,
                         kind="ExternalInput").ap()
    ohg = nc.dram_tensor("ohg", [128, TILES * gslot], f32r,
                         kind="ExternalInput").ap()
    outc = nc.dram_tensor("outc", [TILES, 128, FEAT_DIM], f32,
                          kind="ExternalOutput").ap()
    lossp = nc.dram_tensor("lossp", [gslot, 1], f32,
                           kind="ExternalOutput").ap()

    with tile.TileContext(nc) as tc, ExitStack() as ctx:
        const = ctx.enter_context(tc.tile_pool(name="const", bufs=1))
        cpool = ctx.enter_context(tc.tile_pool(name="cts", bufs=5))
        fpool = ctx.enter_context(tc.tile_pool(name="fg", bufs=4))
        apool = ctx.enter_context(tc.tile_pool(name="aa", bufs=4))
        opool = ctx.enter_context(tc.tile_pool(name="oo", bufs=5))
        dpool = ctx.enter_context(tc.tile_pool(name="dd", bufs=3))
        spool = ctx.enter_context(tc.tile_pool(name="ss", bufs=2))
        pspool = ctx.enter_context(tc.tile_pool(name="psums", bufs=4, space="PSUM"))
        pgpool = ctx.enter_context(tc.tile_pool(name="psumg", bufs=3, space="PSUM"))
        pcpool = ctx.enter_context(tc.tile_pool(name="psumc", bufs=1, space="PSUM"))

        oh_all = const.tile([gslot, ohcols], fdt, tag="oh")
        nc.sync.dma_start(oh_all[:], ohp[:])
        ones = oh_all[:, ohcols - 1:ohcols]
        ohg_all = const.tile([128, TILES * gslot], f32r, tag="ohg")
        nc.sync.dma_start(ohg_all[:], ohg[:])

        def oh_tile(t):
            return oh_all[:, t * 128:(t + 1) * 128]

        # --- phase 1: per-class counts for all tiles, then scale vectors ---
        pc = pcpool.tile([128, TILES], f32, tag="pc")
        for t in range(TILES):
            nc.tensor.matmul(pc[:, t:t + 1], oh_tile(t), ones,
                             start=True, stop=True)
        counts = const.tile([128, TILES], f32, tag="counts")
        nc.vector.tensor_copy(counts[:], pc[:])
        mask = const.tile([128, TILES], f32, tag="mask")
        nc.vector.tensor_scalar(mask[:], counts[:], 0.5, None, op.is_ge)
        csafe = const.tile([128, TILES], f32, tag="csafe")
        nc.vector.tensor_scalar(csafe[:], counts[:], 1.0, None, op.max)
        rcp = const.tile([128, TILES], f32, tag="rcp")
        nc.vector.reciprocal(rcp[:], csafe[:])
        s1 = const.tile([128, TILES], f32, tag="s1")
        nc.vector.tensor_scalar(s1[:], mask[:], -ALPHA, 1.0, op.mult, op.add)
        s2 = const.tile([128, TILES], f32, tag="s2")
        nc.vector.scalar_tensor_tensor(s2[:], rcp[:], ALPHA, mask[:],
                                       op.mult, op.mult)

        lcol = const.tile([gslot, TILES], f32, tag="lcol")

        # --- phase 2: stream center tiles, one feature group per gp tiles ---
        for g in range(ngrp):
            fg = fpool.tile([gslot, gp * FEAT_DIM], fdt, tag="fg")
            nc.sync.dma_start(fg[:], featp[g])
            for h in range(gp):
                t = g * gp + h
                fgh = fg[:, h * FEAT_DIM:(h + 1) * FEAT_DIM]
                ct = cpool.tile([128, FEAT_DIM], f32r, tag="ct")
                nc.sync.dma_start(ct[:], centers[t])
                ps = pspool.tile([128, FEAT_DIM], f32, tag="ps")
                nc.tensor.matmul(ps[:], oh_tile(t), fgh,
                                 start=True, stop=True)
                psg = pgpool.tile([gslot, FEAT_DIM], f32, tag="psg")
                nc.tensor.matmul(psg[:],
                                 ohg_all[:, t * gslot:(t + 1) * gslot],
                                 ct[:], start=True, stop=True)
                aa = apool.tile([128, FEAT_DIM], f32, tag="aa")
                if h == 0:
                    nc.scalar.activation(aa[:], ct[:].bitcast(f32), AF.Copy,
                                         scale=s1[:, t:t + 1])
                else:
                    nc.vector.tensor_scalar(aa[:], ct[:].bitcast(f32),
                                            s1[:, t:t + 1], None, op.mult)
                ot = opool.tile([128, FEAT_DIM], f32, tag="ot")
                nc.vector.scalar_tensor_tensor(ot[:], ps[:], s2[:, t:t + 1],
                                               aa[:], op.mult, op.add)
                nc.sync.dma_start(outc[t], ot[:])
                dt_ = dpool.tile([gslot, FEAT_DIM], f32, tag="dd")
                nc.vector.tensor_tensor(dt_[:], psg[:], fgh, op.subtract)
                sq = spool.tile([gslot, FEAT_DIM], f32, tag="sq")
                nc.scalar.activation(sq[:], dt_[:], AF.Square,
                                     accum_out=lcol[:, t:t + 1])

        # --- phase 3: fold loss partials to one [gslot,1] vector ---
        lv = const.tile([gslot, 1], f32, tag="lv")
        nc.vector.tensor_reduce(lv[:], lcol[:], AX.X, op.add)
        nc.sync.dma_start(lossp[:], lv[:])

    nc.compile()
    return nc


def _route(features, labels, feat_np_dtype, gp):
    """Host-side all-to-all: per core, group features by class tile and
    build one-hot routing matrices. Returns per-core (featp, ohp, ohg)."""
    ngrp = TILES // gp
    gslot = 128 // gp
    feat_maps = []
    oh_maps = []
    ohg_maps = []
    core = labels // CSH
    within = labels % CSH
    tl = within // 128          # class tile within core
    cc = within % 128           # class row within tile
    for m in range(NCORES):
        sel = np.nonzero(core == m)[0]
        t = tl[sel]
        order = np.argsort(t, kind="stable")
        sel, t = sel[order], t[order]
        c = cc[sel]
        # rank within each tile
        slot = np.arange(len(t)) - np.searchsorted(t, t)
        if len(slot) and slot.max() >= gslot:
            raise OverflowError("tile feature count exceeds slots")
        g, h = np.divmod(t, gp)
        featp = np.zeros((ngrp, gslot, gp, FEAT_DIM), dtype=feat_np_dtype)
        featp[g, slot, h] = features[sel]
        featp = featp.reshape(ngrp, gslot, gp * FEAT_DIM)
        ohp = np.zeros((gslot, TILES * 128 + 1), dtype=feat_np_dtype)
        ohp[slot, t * 128 + c] = 1.0
        ohp[:, -1] = 1.0
        ohg = np.zeros((128, TILES * gslot), dtype=np.float32)
        ohg[c, t * gslot + slot] = 1.0
        feat_maps.append(featp)
        oh_maps.append(ohp)
        ohg_maps.append(ohg)
    return feat_maps, oh_maps, ohg_maps


def _run(features, labels, centers, feat_np_dtype=None, trace=False):
    import ml_dtypes
    from concourse.bass_utils import run_bass_kernel_spmd

    if feat_np_dtype is None:
        feat_np_dtype = ml_dtypes.bfloat16

    features = np.ascontiguousarray(np.asarray(features, dtype=np.float32))
    labels = np.asarray(labels).astype(np.int64)
    centers = np.ascontiguousarray(np.asarray(centers, dtype=np.float32))

    centers_pad = np.zeros((CPAD, FEAT_DIM), dtype=np.float32)
    centers_pad[:NUM_CLASSES] = centers
    centers_pad = centers_pad.reshape(NCORES, TILES, 128, FEAT_DIM)

    try:
        gp = 2
        feat_maps, oh_maps, ohg_maps = _route(features, labels,
                                              feat_np_dtype, gp)
    except OverflowError:
        gp = 1
        feat_maps, oh_maps, ohg_maps = _route(features, labels,
                                              feat_np_dtype, gp)

    key = (np.dtype(feat_np_dtype).name, gp)
    if key not in _CACHE:
        _CACHE[key] = _build(feat_np_dtype, gp)
    nc = _CACHE[key]

    in_maps = [
        {"centers": np.ascontiguousarray(centers_pad[m]),
         "featp": feat_maps[m], "ohp": oh_maps[m], "ohg": ohg_maps[m]}
        for m in range(NCORES)
    ]
    res = run_bass_kernel_spmd(nc, in_maps, list(range(NCORES)), trace=trace)

    new_centers = np.concatenate(
        [res.results[m]["outc"].reshape(CSH, FEAT_DIM) for m in range(NCORES)],
        axis=0)[:NUM_CLASSES]
    loss = np.float32(
        sum(np.float64(res.results[m]["lossp"]).sum() for m in range(NCORES))
        / BATCH)
    return (loss, new_centers), res


def kernel(features, labels, centers):
    out, _ = _run(features, labels, centers)
    return out
